# revision 6
# baseline (speedup 1.0000x reference)
"""ComplexMultiheadAttention on 8 Trainium2 NeuronCores.

Sharding: core c handles batch b = c//4 and the 4 heads [4*(c%4), 4*(c%4)+4).

v2 design:
- All tensor-engine operands in bf16 (same PE rate as f32r, half the bytes).
- Q/K projections and the O projection use Karatsuba complex multiplication:
  3 real matmuls (t1=zr@wr, t2=zi@wi, t3=(zr+zi)@(wr+wi)) instead of 4;
  real = t1-t2, imag = t3-t1-t2, combined on DVE/GpSimd (one PSUM operand is
  staged through a Scalar-engine copy because DVE ops may read only one PSUM
  input). The host ships zs = zr+zi and ws = wr+wi, so no extra on-chip adds.
- V projection stays as one fused K=2048 matmul over [zr|zi] (its seq-major
  output layout does not admit the 512-wide Karatsuba strips).
- Softmax rowsums ride DVE + GpSimd strip adds (f32 accumulation) with a
  single ones-matmul fold per (head, q-block); reciprocal uses the fast
  ~18-bit DVE approximation; exp is done once per kt-PAIR on [128,1024] PSUM.
- Phase 3 (O projection) is interleaved per q-block with phase 2 so the PE
  never drains; O-proj inputs are normalized AV tiles regrouped r/i via
  SBUF->SBUF DMA (DVE cannot shift partitions).
- Q-bias, K-bias applied in the combine; V-bias folds into the host-side
  output bias (softmax rows sum to 1); output partials are summed on host.
"""

import os
import sys

import numpy as np
import ml_dtypes

sys.path.insert(0, "/opt/trn_rl_repo")

import concourse.bass as bass
import concourse.bacc as bacc_mod
import concourse.mybir as mybir
from concourse.bass_utils import run_bass_kernel_spmd
from concourse.tile import TileContext

try:  # tracing needs antenv.axon_hooks (test harness injects it)
    import antenv.axon_hooks  # noqa: F401
except ImportError:
    os.environ.setdefault("BASS_NEVER_TRACE", "1")

B, L, D, NH = 2, 2048, 1024, 16
HD = D // NH  # 64
N_CORES = 8
NHL = 4  # heads per core
CH2 = 2 * NHL * HD  # 512 packed channels for fused V
F2 = 2 * D
SCALE = 1.0 / 8.0  # 1/sqrt(HD)
BF16NP = ml_dtypes.bfloat16

F32 = mybir.dt.float32
F32R = mybir.dt.float32r
BF16 = mybir.dt.bfloat16
AF = mybir.ActivationFunctionType
ALU = mybir.AluOpType

NSB = 4
SBW = L // NSB  # 512 seq cols per block


def _build_nc():
    nc = bacc_mod.Bacc(None, target_bir_lowering=False, debug=False)
    z3t = nc.declare_dram_parameter("z3t", [3 * D, L], BF16, isOutput=False)
    wdecl = {}
    for p in ("q", "k"):
        for t in (1, 2, 3):
            wdecl[f"w{p}{t}"] = nc.declare_dram_parameter(
                f"w{p}{t}", [D, 256], BF16, isOutput=False
            )
        for s in ("r", "i"):
            wdecl[f"c{p}{s}"] = nc.declare_dram_parameter(
                f"c{p}{s}", [256], F32, isOutput=False
            )
    wv = nc.declare_dram_parameter("wv", [F2, CH2], BF16, isOutput=False)
    for t in (1, 2, 3):
        wdecl[f"wo{t}"] = nc.declare_dram_parameter(
            f"wo{t}", [256, D], BF16, isOutput=False
        )
    pr = nc.declare_dram_parameter("pr", [L, D], BF16, isOutput=True)
    pi = nc.declare_dram_parameter("pi", [L, D], BF16, isOutput=True)

    with TileContext(nc) as tc:
        with tc.tile_pool(name="pers", bufs=1) as pers:
            # ---- persistent tiles ----
            qt = [
                pers.tile([128, L], BF16, tag=f"qt{h}", name=f"qt{h}")
                for h in range(NHL)
            ]
            kt = [
                pers.tile([128, L], BF16, tag=f"kt{h}", name=f"kt{h}")
                for h in range(NHL)
            ]
            v4 = pers.tile([128, 16, CH2], BF16, tag="v4")
            ones_f = pers.tile([128, 1], F32, tag="ones_f")
            nc.vector.memset(ones_f[:], 1.0)
            ones = pers.tile([128, 1], F32R, tag="ones")
            nc.scalar.activation(ones[:], ones_f[:], AF.Copy)
            onesr_f = pers.tile([1, 128], F32, tag="onesr_f")
            nc.vector.memset(onesr_f[:], 1.0)
            warm = pers.tile([128, 8], F32, tag="warm")

            # ---------- Phase 1: QKV projections ----------
            with (
                tc.tile_pool(name="w1p", bufs=1) as wpool,
                tc.tile_pool(name="z3p", bufs=2) as zpool,
                tc.tile_pool(name="st1", bufs=3) as st1,
                tc.tile_pool(name="ps_kar", bufs=2, space="PSUM") as ps_kar,
                tc.tile_pool(name="ps_v", bufs=2, space="PSUM") as ps_v,
            ):
                z3_first = zpool.tile([128, 24, SBW], BF16, tag="z3")

                def z3_dma(tile, sb):
                    for ftg in range(6):  # 6 DMAs of 4 chunks each
                        nc.sync.dma_start(
                            tile[:, ftg * 4 : (ftg + 1) * 4, :],
                            z3t[
                                ftg * 512 : (ftg + 1) * 512,
                                sb * SBW : (sb + 1) * SBW,
                            ].rearrange("(t p) s -> p t s", p=128),
                        )

                z3_dma(z3_first, 0)
                wsb = {}
                for p in ("q", "k"):
                    for t in (1, 2, 3):
                        wsb[f"{p}{t}"] = wpool.tile(
                            [128, 8, 256], BF16, tag=f"w{p}{t}", name=f"w{p}{t}"
                        )
                        nc.sync.dma_start(
                            wsb[f"{p}{t}"][:],
                            wdecl[f"w{p}{t}"][:].rearrange("(t p) s -> p t s", p=128),
                        )
                    for s in ("r", "i"):
                        wsb[f"c{p}{s}"] = wpool.tile(
                            [128, 2], F32, tag=f"c{p}{s}", name=f"c{p}{s}"
                        )
                        nc.sync.dma_start(
                            wsb[f"c{p}{s}"][:],
                            wdecl[f"c{p}{s}"][:].rearrange("(t p) -> p t", p=128),
                        )
                wv_sb = wpool.tile([128, 16, CH2], BF16, tag="wv")
                nc.sync.dma_start(
                    wv_sb[:], wv[:].rearrange("(t p) s -> p t s", p=128)
                )

                for sb in range(NSB):
                    if sb == 0:
                        z3_sb = z3_first
                    else:
                        z3_sb = zpool.tile([128, 24, SBW], BF16, tag="z3")
                        z3_dma(z3_sb, sb)
                    for p in ("q", "k"):
                        dst = qt if p == "q" else kt
                        for ct in range(2):
                            t1 = ps_kar.tile([128, SBW], F32, tag="t1")
                            t2 = ps_kar.tile([128, SBW], F32, tag="t2")
                            t3 = ps_kar.tile([128, SBW], F32, tag="t3")
                            for term, ps in ((1, t1), (2, t2), (3, t3)):
                                wt = wsb[f"{p}{term}"]
                                zoff = (term - 1) * 8
                                for ft in range(8):
                                    nc.tensor.matmul(
                                        ps[:],
                                        lhsT=wt[:, ft, ct * 128 : (ct + 1) * 128],
                                        rhs=z3_sb[:, zoff + ft, :],
                                        start=(ft == 0),
                                        stop=(ft == 7),
                                    )
                            t2_sb = st1.tile([128, SBW], F32R, tag="t2sb")
                            nc.scalar.activation(t2_sb[:], t2[:], AF.Copy)
                            cbr = wsb[f"c{p}r"][:, ct : ct + 1]
                            cbi = wsb[f"c{p}i"][:, ct : ct + 1]
                            reX = st1.tile([128, SBW], BF16, tag="reX")
                            u = st1.tile([128, SBW], F32R, tag="u")
                            imX = st1.tile([128, SBW], BF16, tag="imX")
                            with nc.allow_low_precision(reason="bf16 qkv"):
                                # real = (t1 + cbr) - t2
                                nc.vector.scalar_tensor_tensor(
                                    reX[:], t1[:], cbr, t2_sb[:],
                                    op0=ALU.add, op1=ALU.subtract,
                                )
                                # u = (t1 - cbi) + t2 ; imag = t3 - u
                                # (GpSimd cannot read PSUM, so all on DVE)
                                nc.vector.scalar_tensor_tensor(
                                    u[:], t1[:], cbi, t2_sb[:],
                                    op0=ALU.subtract, op1=ALU.add,
                                )
                                nc.vector.tensor_sub(imX[:], t3[:], u[:])
                            ha, hb = 2 * ct, 2 * ct + 1
                            cols = slice(sb * SBW, (sb + 1) * SBW)
                            nc.sync.dma_start(dst[ha][0:64, cols], reX[0:64, :])
                            nc.sync.dma_start(dst[hb][0:64, cols], reX[64:128, :])
                            nc.sync.dma_start(dst[ha][64:128, cols], imX[0:64, :])
                            nc.sync.dma_start(dst[hb][64:128, cols], imX[64:128, :])
                    for st in range(SBW // 128):
                        psv = ps_v.tile([128, CH2], F32, tag="psv")
                        for ft in range(16):
                            nc.tensor.matmul(
                                psv[:],
                                lhsT=z3_sb[:, ft, st * 128 : (st + 1) * 128],
                                rhs=wv_sb[:, ft, :],
                                start=(ft == 0),
                                stop=(ft == 15),
                            )
                        nc.scalar.activation(
                            v4[:, sb * 4 + st, :], psv[:], AF.Copy
                        )

            # warm up GpSimd custom-instruction library before phase 2
            nc.gpsimd.partition_broadcast(warm[:], onesr_f[0:1, 0:8])

            # ---------- Phase 2+3: attention with interleaved O-projection ----
            with (
                tc.tile_pool(name="wo_p", bufs=1) as wopool,
                tc.tile_pool(name="otp", bufs=1) as otpool,
                tc.tile_pool(name="pp", bufs=4) as ppool,
                tc.tile_pool(name="accs", bufs=2) as apool,
                tc.tile_pool(name="sm2", bufs=4) as spool,
                tc.tile_pool(name="st3", bufs=3) as st3,
                tc.tile_pool(name="ps_big", bufs=2, space="PSUM") as ps_big,
                tc.tile_pool(name="ps_av", bufs=2, space="PSUM") as ps_av,
                tc.tile_pool(name="ps_sum", bufs=2, space="PSUM") as ps_sum,
            ):
                wo_sb = {}
                for t in (1, 2, 3):
                    wo_sb[t] = wopool.tile(
                        [128, 2, D], BF16, tag=f"wo{t}", name=f"wo{t}"
                    )
                    nc.sync.dma_start(
                        wo_sb[t][:],
                        wdecl[f"wo{t}"][:].rearrange("(t p) s -> p t s", p=128),
                    )
                # OT tiles per qb: [r|i|s] x [chunk a (h0,h1) | chunk b (h2,h3)]
                ot = {}
                for qb in range(4):
                    for kind in ("r", "i", "s"):
                        for chunk in range(2):
                            ot[(qb, kind, chunk)] = otpool.tile(
                                [128, SBW], BF16,
                                tag=f"ot{qb}{kind}{chunk}",
                                name=f"ot{qb}{kind}{chunk}",
                            )

                NAHEAD = 2  # pair lookahead
                p_tiles = {}

                def emit_pair(h, qb, kp):
                    sp = ps_big.tile([128, 1024], F32, tag="pair")
                    for j in range(2):
                        nc.tensor.matmul(
                            sp[:, j * 512 : (j + 1) * 512],
                            lhsT=kt[h][:, (2 * kp + j) * 128 : (2 * kp + j + 1) * 128],
                            rhs=qt[h][:, qb * 512 : (qb + 1) * 512],
                            start=True,
                            stop=True,
                        )
                    p_sb = ppool.tile([128, 1024], BF16, tag="p")
                    nc.scalar.activation(p_sb[:], sp[:], AF.Exp, scale=SCALE)
                    p_tiles[kp] = p_sb

                pending = []

                def flush_tail(limit):
                    while len(pending) > limit:
                        av, acc_a, acc_b, h, qb = pending.pop(0)
                        ssum = ps_sum.tile([1, 512], F32, tag="ssum")
                        nc.tensor.matmul(
                            ssum[:], lhsT=ones[:, 0:1], rhs=acc_a[:],
                            start=True, stop=False,
                        )
                        nc.tensor.matmul(
                            ssum[:], lhsT=ones[:, 0:1], rhs=acc_b[:],
                            start=False, stop=True,
                        )
                        recip = spool.tile([1, 512], F32, tag="recip")
                        nc.vector.reciprocal_approx_fast(recip[:], ssum[:])
                        rb = spool.tile([128, 512], F32, tag="rb")
                        nc.gpsimd.partition_broadcast(rb[:], recip[:])
                        avn = spool.tile([128, 512], BF16, tag="avn")
                        with nc.allow_low_precision(reason="bf16 attn out"):
                            nc.vector.tensor_mul(avn[:], av[:], rb[:])
                        chunk = h // 2
                        prt = slice((h % 2) * 64, (h % 2) * 64 + 64)
                        nc.sync.dma_start(ot[(qb, "r", chunk)][prt, :], avn[0:64, :])
                        nc.sync.dma_start(ot[(qb, "i", chunk)][prt, :], avn[64:128, :])
                        with nc.allow_low_precision(reason="bf16 ot sum"):
                            nc.gpsimd.tensor_add(
                                ot[(qb, "s", chunk)][prt, :],
                                ot[(qb, "r", chunk)][prt, :],
                                ot[(qb, "i", chunk)][prt, :],
                            )

                def attention_block(h, qb):
                    av = ps_av.tile([128, 512], F32, tag="av")
                    acc_a = apool.tile([128, 512], F32R, tag="acc_a")
                    acc_b = apool.tile([128, 512], F32R, tag="acc_b")
                    for kp in range(8):
                        if kp == 0:
                            for k2 in range(NAHEAD):
                                emit_pair(h, qb, k2)
                        p_sb = p_tiles.pop(kp)
                        for j in range(2):
                            kt_i = 2 * kp + j
                            nc.tensor.matmul(
                                av[:],
                                lhsT=v4[:, kt_i, 128 * h : 128 * (h + 1)],
                                rhs=p_sb[:, j * 512 : (j + 1) * 512],
                                start=(kt_i == 0),
                                stop=(kt_i == 15),
                            )
                        if kp + NAHEAD < 8:
                            emit_pair(h, qb, kp + NAHEAD)
                        if kp == 1:
                            flush_tail(0)
                        # rowsum: DVE owns strips 0..7 (kp<4), gpsimd 8..15
                        eng = nc.vector if kp < 4 else nc.gpsimd
                        acc = acc_a if kp < 4 else acc_b
                        first = kp % 4 == 0
                        with nc.allow_low_precision(reason="f32r rowsum"):
                            if first:
                                eng.tensor_add(
                                    acc[:], p_sb[:, 0:512], p_sb[:, 512:1024]
                                )
                            else:
                                eng.tensor_add(acc[:], acc[:], p_sb[:, 0:512])
                                eng.tensor_add(acc[:], acc[:], p_sb[:, 512:1024])
                    pending.append((av, acc_a, acc_b, h, qb))

                def oproj_group(qb, qt_local, nb):
                    big = ps_big.tile([128, 1024], F32, tag="pair")
                    t3p = ps_av.tile([128, 512], F32, tag="av")
                    t1p = big[:, 0:512]
                    t2p = big[:, 512:1024]
                    qs = slice(qt_local * 128, (qt_local + 1) * 128)
                    ns = slice(nb * 512, (nb + 1) * 512)
                    for chunk in range(2):
                        nc.tensor.matmul(
                            t1p, lhsT=ot[(qb, "r", chunk)][:, qs],
                            rhs=wo_sb[1][:, chunk, ns],
                            start=(chunk == 0), stop=(chunk == 1),
                        )
                        nc.tensor.matmul(
                            t2p, lhsT=ot[(qb, "i", chunk)][:, qs],
                            rhs=wo_sb[2][:, chunk, ns],
                            start=(chunk == 0), stop=(chunk == 1),
                        )
                        nc.tensor.matmul(
                            t3p[:], lhsT=ot[(qb, "s", chunk)][:, qs],
                            rhs=wo_sb[3][:, chunk, ns],
                            start=(chunk == 0), stop=(chunk == 1),
                        )
                    t1_sb = st3.tile([128, 512], F32R, tag="t1sb3")
                    nc.scalar.activation(t1_sb[:], t1p, AF.Copy)
                    t2_sb = st3.tile([128, 512], F32R, tag="t2sb3")
                    nc.scalar.activation(t2_sb[:], t2p, AF.Copy)
                    outr = st3.tile([128, 512], BF16, tag="outr")
                    u3 = st3.tile([128, 512], F32R, tag="u3")
                    outi = st3.tile([128, 512], BF16, tag="outi")
                    with nc.allow_low_precision(reason="bf16 out"):
                        nc.vector.tensor_sub(outr[:], t1p, t2_sb[:])
                        nc.gpsimd.tensor_add(u3[:], t1_sb[:], t2_sb[:])
                        nc.vector.tensor_sub(outi[:], t3p[:], u3[:])
                    qrow = qb * 512 + qt_local * 128
                    nc.sync.dma_start(pr[qrow : qrow + 128, ns], outr[:])
                    nc.sync.dma_start(pi[qrow : qrow + 128, ns], outi[:])

                for qb in range(4):
                    for h in range(NHL):
                        attention_block(h, qb)
                    flush_tail(0)
                    for qt_local in range(4):
                        for nb in range(2):
                            oproj_group(qb, qt_local, nb)
    if not nc.is_finalized():
        nc.finalize()
    return nc


_NC = None


def _get_nc():
    global _NC
    if _NC is None:
        _NC = _build_nc()
    return _NC


def _prep(inputs):
    f = lambda k: np.asarray(inputs[k], np.float32)
    zr, zi = f("zr"), f("zi")
    w = {n: f(n) for n in inputs if n not in ("zr", "zi")}

    z3t = []
    for b in range(B):
        zs = zr[b] + zi[b]
        z3t.append(
            np.concatenate([zr[b].T, zi[b].T, zs.T], axis=0).astype(BF16NP)
        )

    in_maps = []
    for c in range(N_CORES):
        b, hg = c // 4, c % 4
        m = {"z3t": z3t[b]}
        Ch = np.arange(hg * 256, (hg + 1) * 256)
        for name in ("q", "k"):
            wr = w[f"w{name}_r"][Ch, :]
            wi = w[f"w{name}_i"][Ch, :]
            m[f"w{name}1"] = np.ascontiguousarray(wr.T).astype(BF16NP)
            m[f"w{name}2"] = np.ascontiguousarray(wi.T).astype(BF16NP)
            m[f"w{name}3"] = np.ascontiguousarray((wr + wi).T).astype(BF16NP)
            br, bi = w[f"b{name}_r"][Ch], w[f"b{name}_i"][Ch]
            m[f"c{name}r"] = (br - bi).astype(np.float32)
            m[f"c{name}i"] = (br + bi).astype(np.float32)
        wvr, wvi = w["wv_r"], w["wv_i"]
        wvcat = np.empty((F2, CH2), np.float32)
        for l in range(NHL):
            Chh = np.arange((hg * 4 + l) * HD, (hg * 4 + l + 1) * HD)
            s = l * 128
            wvcat[:D, s : s + 64] = wvr[Chh, :].T
            wvcat[D:, s : s + 64] = -wvi[Chh, :].T
            wvcat[:D, s + 64 : s + 128] = wvi[Chh, :].T
            wvcat[D:, s + 64 : s + 128] = wvr[Chh, :].T
        m["wv"] = wvcat.astype(BF16NP)
        wor = w["wo_r"][:, Ch]  # [1024 outd, 256 local ch]
        woi = w["wo_i"][:, Ch]
        m["wo1"] = np.ascontiguousarray(wor.T).astype(BF16NP)
        m["wo2"] = np.ascontiguousarray(woi.T).astype(BF16NP)
        m["wo3"] = np.ascontiguousarray((wor + woi).T).astype(BF16NP)
        in_maps.append(m)

    # exact host-side bias: V-bias folds through softmax (rows sum to 1)
    cvr = (w["bv_r"] - w["bv_i"]).astype(np.float64)
    cvi = (w["bv_r"] + w["bv_i"]).astype(np.float64)
    wo_r = w["wo_r"].astype(np.float64)
    wo_i = w["wo_i"].astype(np.float64)
    br_total = wo_r @ cvr - wo_i @ cvi + w["bo_r"] - w["bo_i"]
    bi_total = wo_r @ cvi + wo_i @ cvr + w["bo_r"] + w["bo_i"]
    return in_maps, br_total.astype(np.float32), bi_total.astype(np.float32)


LAST_RESULTS = None


def kernel(**inputs):
    global LAST_RESULTS
    nc = _get_nc()
    in_maps, br_total, bi_total = _prep(inputs)
    res = run_bass_kernel_spmd(nc, in_maps, core_ids=list(range(N_CORES)))
    LAST_RESULTS = res
    out_r = np.zeros((B, L, D), np.float32)
    out_i = np.zeros((B, L, D), np.float32)
    for c in range(N_CORES):
        out_r[c // 4] += np.asarray(res.results[c]["pr"], np.float32)
        out_i[c // 4] += np.asarray(res.results[c]["pi"], np.float32)
    out_r += br_total[None, None, :]
    out_i += bi_total[None, None, :]
    return out_r, out_i


# revision 10
# speedup vs baseline: 1.0863x; 1.0863x over previous
"""ComplexMultiheadAttention on 8 Trainium2 NeuronCores.

Sharding: core c handles batch b = c//4 and the 4 heads [4*(c%4), 4*(c%4)+4).

v2 design:
- All tensor-engine operands in bf16 (same PE rate as f32r, half the bytes).
- Q/K projections and the O projection use Karatsuba complex multiplication:
  3 real matmuls (t1=zr@wr, t2=zi@wi, t3=(zr+zi)@(wr+wi)) instead of 4;
  real = t1-t2, imag = t3-t1-t2, combined on DVE/GpSimd (one PSUM operand is
  staged through a Scalar-engine copy because DVE ops may read only one PSUM
  input). The host ships zs = zr+zi and ws = wr+wi, so no extra on-chip adds.
- V projection stays as one fused K=2048 matmul over [zr|zi] (its seq-major
  output layout does not admit the 512-wide Karatsuba strips).
- Softmax rowsums ride DVE + GpSimd strip adds (f32 accumulation) with a
  single ones-matmul fold per (head, q-block); reciprocal uses the fast
  ~18-bit DVE approximation; exp is done once per kt-PAIR on [128,1024] PSUM.
- Phase 3 (O projection) is interleaved per q-block with phase 2 so the PE
  never drains; O-proj inputs are normalized AV tiles regrouped r/i via
  SBUF->SBUF DMA (DVE cannot shift partitions).
- Q-bias, K-bias applied in the combine; V-bias folds into the host-side
  output bias (softmax rows sum to 1); output partials are summed on host.
"""

import os
import sys

import numpy as np
import ml_dtypes

sys.path.insert(0, "/opt/trn_rl_repo")

import concourse.bass as bass
import concourse.bacc as bacc_mod
import concourse.mybir as mybir
from concourse.bass_utils import run_bass_kernel_spmd
from concourse.tile import TileContext

try:  # tracing needs antenv.axon_hooks (test harness injects it)
    import antenv.axon_hooks  # noqa: F401
except ImportError:
    os.environ.setdefault("BASS_NEVER_TRACE", "1")

B, L, D, NH = 2, 2048, 1024, 16
HD = D // NH  # 64
N_CORES = 8
NHL = 4  # heads per core
CH2 = 2 * NHL * HD  # 512 packed channels for fused V
F2 = 2 * D
SCALE = 1.0 / 8.0  # 1/sqrt(HD)
BF16NP = ml_dtypes.bfloat16

F32 = mybir.dt.float32
F32R = mybir.dt.float32r
BF16 = mybir.dt.bfloat16
AF = mybir.ActivationFunctionType
ALU = mybir.AluOpType

NSB = 4
SBW = L // NSB  # 512 seq cols per block


def _build_nc():
    nc = bacc_mod.Bacc(None, target_bir_lowering=False, debug=False)
    z3t = nc.declare_dram_parameter("z3t", [3 * D, L], BF16, isOutput=False)
    wdecl = {}
    for p in ("q", "k"):
        for t in (1, 2, 3):
            wdecl[f"w{p}{t}"] = nc.declare_dram_parameter(
                f"w{p}{t}", [D, 256], BF16, isOutput=False
            )
        for s in ("r", "i"):
            wdecl[f"c{p}{s}"] = nc.declare_dram_parameter(
                f"c{p}{s}", [256], F32, isOutput=False
            )
    wv = nc.declare_dram_parameter("wv", [F2, CH2], BF16, isOutput=False)
    wdecl["wor"] = nc.declare_dram_parameter("wor", [CH2, D], BF16, isOutput=False)
    wdecl["woi"] = nc.declare_dram_parameter("woi", [CH2, D], BF16, isOutput=False)
    pr = nc.declare_dram_parameter("pr", [L, D], BF16, isOutput=True)
    pi = nc.declare_dram_parameter("pi", [L, D], BF16, isOutput=True)

    with TileContext(nc) as tc:
        with tc.tile_pool(name="pers", bufs=1) as pers:
            # ---- persistent tiles ----
            qt = [
                pers.tile([128, L], BF16, tag=f"qt{h}", name=f"qt{h}")
                for h in range(NHL)
            ]
            kt = [
                pers.tile([128, L], BF16, tag=f"kt{h}", name=f"kt{h}")
                for h in range(NHL)
            ]
            v4 = pers.tile([128, 16, CH2], BF16, tag="v4")
            ones_f = pers.tile([128, 1], F32, tag="ones_f")
            nc.vector.memset(ones_f[:], 1.0)
            ones = pers.tile([128, 1], F32R, tag="ones")
            nc.scalar.activation(ones[:], ones_f[:], AF.Copy)
            onesr_f = pers.tile([1, 128], F32, tag="onesr_f")
            nc.vector.memset(onesr_f[:], 1.0)
            warm = pers.tile([128, 8], F32, tag="warm")

            # ---------- Phase 1: QKV projections ----------
            with (
                tc.tile_pool(name="w1p", bufs=1) as wpool,
                tc.tile_pool(name="z3p", bufs=2) as zpool,
                tc.tile_pool(name="st1", bufs=3) as st1,
                tc.tile_pool(name="ps_kar", bufs=2, space="PSUM") as ps_kar,
                tc.tile_pool(name="ps_v", bufs=2, space="PSUM") as ps_v,
            ):
                z3_first = zpool.tile([128, 24, SBW], BF16, tag="z3")

                def z3_dma(tile, sb):
                    for ftg in range(6):  # 6 DMAs of 4 chunks each
                        nc.sync.dma_start(
                            tile[:, ftg * 4 : (ftg + 1) * 4, :],
                            z3t[
                                ftg * 512 : (ftg + 1) * 512,
                                sb * SBW : (sb + 1) * SBW,
                            ].rearrange("(t p) s -> p t s", p=128),
                        )

                z3_dma(z3_first, 0)
                wsb = {}
                for p in ("q", "k"):
                    for t in (1, 2, 3):
                        wsb[f"{p}{t}"] = wpool.tile(
                            [128, 8, 256], BF16, tag=f"w{p}{t}", name=f"w{p}{t}"
                        )
                        nc.sync.dma_start(
                            wsb[f"{p}{t}"][:],
                            wdecl[f"w{p}{t}"][:].rearrange("(t p) s -> p t s", p=128),
                        )
                    for s in ("r", "i"):
                        wsb[f"c{p}{s}"] = wpool.tile(
                            [128, 2], F32, tag=f"c{p}{s}", name=f"c{p}{s}"
                        )
                        nc.sync.dma_start(
                            wsb[f"c{p}{s}"][:],
                            wdecl[f"c{p}{s}"][:].rearrange("(t p) -> p t", p=128),
                        )
                wv_sb = wpool.tile([128, 16, CH2], BF16, tag="wv")
                nc.sync.dma_start(
                    wv_sb[:], wv[:].rearrange("(t p) s -> p t s", p=128)
                )

                for sb in range(NSB):
                    if sb == 0:
                        z3_sb = z3_first
                    else:
                        z3_sb = zpool.tile([128, 24, SBW], BF16, tag="z3")
                        z3_dma(z3_sb, sb)
                    for p in ("q", "k"):
                        dst = qt if p == "q" else kt
                        for ct in range(2):
                            t1 = ps_kar.tile([128, SBW], F32, tag="t1")
                            t2 = ps_kar.tile([128, SBW], F32, tag="t2")
                            t3 = ps_kar.tile([128, SBW], F32, tag="t3")
                            for term, ps in ((1, t1), (2, t2), (3, t3)):
                                wt = wsb[f"{p}{term}"]
                                zoff = (term - 1) * 8
                                for ft in range(8):
                                    nc.tensor.matmul(
                                        ps[:],
                                        lhsT=wt[:, ft, ct * 128 : (ct + 1) * 128],
                                        rhs=z3_sb[:, zoff + ft, :],
                                        start=(ft == 0),
                                        stop=(ft == 7),
                                    )
                            t2_sb = st1.tile([128, SBW], F32R, tag="t2sb")
                            nc.scalar.activation(t2_sb[:], t2[:], AF.Copy)
                            cbr = wsb[f"c{p}r"][:, ct : ct + 1]
                            cbi = wsb[f"c{p}i"][:, ct : ct + 1]
                            reX = st1.tile([128, SBW], BF16, tag="reX")
                            u = st1.tile([128, SBW], F32R, tag="u")
                            imX = st1.tile([128, SBW], BF16, tag="imX")
                            with nc.allow_low_precision(reason="bf16 qkv"):
                                # real = (t1 + cbr) - t2
                                nc.vector.scalar_tensor_tensor(
                                    reX[:], t1[:], cbr, t2_sb[:],
                                    op0=ALU.add, op1=ALU.subtract,
                                )
                                # u = (t1 - cbi) + t2 ; imag = t3 - u
                                # (GpSimd cannot read PSUM, so all on DVE)
                                nc.vector.scalar_tensor_tensor(
                                    u[:], t1[:], cbi, t2_sb[:],
                                    op0=ALU.subtract, op1=ALU.add,
                                )
                                nc.vector.tensor_sub(imX[:], t3[:], u[:])
                            ha, hb = 2 * ct, 2 * ct + 1
                            cols = slice(sb * SBW, (sb + 1) * SBW)
                            nc.sync.dma_start(dst[ha][0:64, cols], reX[0:64, :])
                            nc.sync.dma_start(dst[hb][0:64, cols], reX[64:128, :])
                            nc.sync.dma_start(dst[ha][64:128, cols], imX[0:64, :])
                            nc.sync.dma_start(dst[hb][64:128, cols], imX[64:128, :])
                    for st in range(SBW // 128):
                        psv = ps_v.tile([128, CH2], F32, tag="psv")
                        for ft in range(16):
                            nc.tensor.matmul(
                                psv[:],
                                lhsT=z3_sb[:, ft, st * 128 : (st + 1) * 128],
                                rhs=wv_sb[:, ft, :],
                                start=(ft == 0),
                                stop=(ft == 15),
                            )
                        nc.scalar.activation(
                            v4[:, sb * 4 + st, :], psv[:], AF.Copy
                        )

            # warm up GpSimd custom-instruction library before phase 2
            nc.gpsimd.partition_broadcast(warm[:], onesr_f[0:1, 0:8])

            # ---------- Phase 2+3: attention with interleaved O-projection ----
            # Fused O-projection: OT tiles are the normalized AV tiles directly
            # (per-head [64r|64i] packing), no elementwise combine needed.
            with (
                tc.tile_pool(name="wo_p", bufs=1) as wopool,
                tc.tile_pool(name="otp", bufs=1) as otpool,
                tc.tile_pool(name="pp", bufs=12) as ppool,
                tc.tile_pool(name="tl1", bufs=2) as l1pool,
                tc.tile_pool(name="tl2", bufs=2) as l2pool,
                tc.tile_pool(name="tl3", bufs=2) as l3pool,
                tc.tile_pool(name="accs", bufs=2) as apool,
                tc.tile_pool(name="sm2", bufs=4) as spool,
                tc.tile_pool(name="st3", bufs=4) as st3,
                tc.tile_pool(name="ps_big", bufs=2, space="PSUM") as ps_big,
                tc.tile_pool(name="ps_av", bufs=2, space="PSUM") as ps_av,
                tc.tile_pool(name="ps_sum", bufs=2, space="PSUM") as ps_sum,
            ):
                wor_sb = wopool.tile([128, NHL, D], BF16, tag="wor")
                nc.sync.dma_start(
                    wor_sb[:], wdecl["wor"][:].rearrange("(t p) s -> p t s", p=128)
                )
                woi_sb = wopool.tile([128, NHL, D], BF16, tag="woi")
                nc.sync.dma_start(
                    woi_sb[:], wdecl["woi"][:].rearrange("(t p) s -> p t s", p=128)
                )
                # OT tiles: normalized AV, per (h, qb), [128 ch = 64r|64i, 512 q]
                ot = {}
                for qb in range(4):
                    for h in range(NHL):
                        ot[(h, qb)] = otpool.tile(
                            [128, SBW], BF16, tag=f"ot{h}_{qb}", name=f"ot{h}_{qb}"
                        )

                NAHEAD = 2  # pair lookahead
                p_tiles = {}

                def emit_pair(h, qb, kp):
                    sp = ps_big.tile([128, 1024], F32, tag="pair")
                    for j in range(2):
                        nc.tensor.matmul(
                            sp[:, j * 512 : (j + 1) * 512],
                            lhsT=kt[h][:, (2 * kp + j) * 128 : (2 * kp + j + 1) * 128],
                            rhs=qt[h][:, qb * 512 : (qb + 1) * 512],
                            start=True,
                            stop=True,
                        )
                    p_sb = ppool.tile([128, 1024], BF16, tag="p")
                    nc.scalar.activation(p_sb[:], sp[:], AF.Exp, scale=SCALE)
                    p_tiles[kp] = p_sb
                    return p_sb

                pending = []

                def flush_tail(limit):
                    while len(pending) > limit:
                        av, acc, h, qb = pending.pop(0)
                        ssum = ps_sum.tile([1, 512], F32, tag="ssum")
                        nc.tensor.matmul(
                            ssum[:], lhsT=ones[:, 0:1], rhs=acc[:],
                            start=True, stop=True,
                        )
                        recip = spool.tile([1, 512], F32, tag="recip")
                        nc.vector.reciprocal_approx_fast(recip[:], ssum[:])
                        rb = spool.tile([128, 512], F32, tag="rb")
                        nc.gpsimd.partition_broadcast(rb[:], recip[:])
                        with nc.allow_low_precision(reason="bf16 attn out"):
                            nc.vector.tensor_mul(ot[(h, qb)][:], av[:], rb[:])

                def attention_block(h, qb):
                    av = ps_av.tile([128, 512], F32, tag="av")
                    strips = []
                    for kp in range(8):
                        if kp == 0:
                            for k2 in range(NAHEAD):
                                strips.append(emit_pair(h, qb, k2))
                        if kp + NAHEAD < 8:
                            strips.append(emit_pair(h, qb, kp + NAHEAD))
                        p_sb = p_tiles.pop(kp)
                        for j in range(2):
                            kt_i = 2 * kp + j
                            nc.tensor.matmul(
                                av[:],
                                lhsT=v4[:, kt_i, 128 * h : 128 * (h + 1)],
                                rhs=p_sb[:, j * 512 : (j + 1) * 512],
                                start=(kt_i == 0),
                                stop=(kt_i == 15),
                            )
                        if kp == 1:
                            flush_tail(0)
                    # parallel tree rowsum over the 8 pair tiles ([128,1024] ops)
                    # DVE: L1 pairs 0,1 + L2a; gpsimd: L1 pairs 2,3 + L2b; then
                    # gpsimd L3, DVE final half-add (f32r for the PE fold).
                    l1 = []
                    with nc.allow_low_precision(reason="bf16 rowsum tree"):
                        for i in range(4):
                            t = l1pool.tile([128, 1024], BF16, tag=f"l1_{i}",
                                            name=f"l1_{i}")
                            eng = nc.vector if i < 2 else nc.gpsimd
                            eng.tensor_add(t[:], strips[2 * i][:], strips[2 * i + 1][:])
                            l1.append(t)
                        l2a = l2pool.tile([128, 1024], BF16, tag="l2a")
                        nc.vector.tensor_add(l2a[:], l1[0][:], l1[1][:])
                        l2b = l2pool.tile([128, 1024], BF16, tag="l2b")
                        nc.gpsimd.tensor_add(l2b[:], l1[2][:], l1[3][:])
                        l3 = l3pool.tile([128, 1024], BF16, tag="l3")
                        nc.gpsimd.tensor_add(l3[:], l2a[:], l2b[:])
                        acc = apool.tile([128, 512], F32R, tag="acc")
                        nc.vector.tensor_add(acc[:], l3[:, 0:512], l3[:, 512:1024])
                    pending.append((av, acc, h, qb))

                def oproj_group(qb, qt_local, nb):
                    big = ps_big.tile([128, 1024], F32, tag="pair")
                    rp = big[:, 0:512]
                    ip = big[:, 512:1024]
                    qs = slice(qt_local * 128, (qt_local + 1) * 128)
                    ns = slice(nb * 512, (nb + 1) * 512)
                    for h in range(NHL):
                        nc.tensor.matmul(
                            rp, lhsT=ot[(h, qb)][:, qs], rhs=wor_sb[:, h, ns],
                            start=(h == 0), stop=(h == NHL - 1),
                        )
                    for h in range(NHL):
                        nc.tensor.matmul(
                            ip, lhsT=ot[(h, qb)][:, qs], rhs=woi_sb[:, h, ns],
                            start=(h == 0), stop=(h == NHL - 1),
                        )
                    outr = st3.tile([128, 512], BF16, tag="outr")
                    nc.scalar.activation(outr[:], rp, AF.Copy)
                    outi = st3.tile([128, 512], BF16, tag="outi")
                    nc.scalar.activation(outi[:], ip, AF.Copy)
                    qrow = qb * 512 + qt_local * 128
                    nc.sync.dma_start(pr[qrow : qrow + 128, ns], outr[:])
                    nc.sync.dma_start(pi[qrow : qrow + 128, ns], outi[:])

                for qb in range(4):
                    for h in range(NHL):
                        attention_block(h, qb)
                    flush_tail(0)
                    for qt_local in range(4):
                        for nb in range(2):
                            oproj_group(qb, qt_local, nb)
    if not nc.is_finalized():
        nc.finalize()
    return nc


_NC = None


def _get_nc():
    global _NC
    if _NC is None:
        _NC = _build_nc()
    return _NC


def _prep(inputs):
    f = lambda k: np.asarray(inputs[k], np.float32)
    zr, zi = f("zr"), f("zi")
    w = {n: f(n) for n in inputs if n not in ("zr", "zi")}

    z3t = []
    for b in range(B):
        zs = zr[b] + zi[b]
        z3t.append(
            np.concatenate([zr[b].T, zi[b].T, zs.T], axis=0).astype(BF16NP)
        )

    in_maps = []
    for c in range(N_CORES):
        b, hg = c // 4, c % 4
        m = {"z3t": z3t[b]}
        Ch = np.arange(hg * 256, (hg + 1) * 256)
        for name in ("q", "k"):
            wr = w[f"w{name}_r"][Ch, :]
            wi = w[f"w{name}_i"][Ch, :]
            m[f"w{name}1"] = np.ascontiguousarray(wr.T).astype(BF16NP)
            m[f"w{name}2"] = np.ascontiguousarray(wi.T).astype(BF16NP)
            m[f"w{name}3"] = np.ascontiguousarray((wr + wi).T).astype(BF16NP)
            br, bi = w[f"b{name}_r"][Ch], w[f"b{name}_i"][Ch]
            m[f"c{name}r"] = (br - bi).astype(np.float32)
            m[f"c{name}i"] = (br + bi).astype(np.float32)
        wvr, wvi = w["wv_r"], w["wv_i"]
        wvcat = np.empty((F2, CH2), np.float32)
        for l in range(NHL):
            Chh = np.arange((hg * 4 + l) * HD, (hg * 4 + l + 1) * HD)
            s = l * 128
            wvcat[:D, s : s + 64] = wvr[Chh, :].T
            wvcat[D:, s : s + 64] = -wvi[Chh, :].T
            wvcat[:D, s + 64 : s + 128] = wvi[Chh, :].T
            wvcat[D:, s + 64 : s + 128] = wvr[Chh, :].T
        m["wv"] = wvcat.astype(BF16NP)
        wo_r, wo_i = w["wo_r"], w["wo_i"]
        wor = np.empty((CH2, D), np.float32)
        woi = np.empty((CH2, D), np.float32)
        for l in range(NHL):
            Chh = np.arange((hg * 4 + l) * HD, (hg * 4 + l + 1) * HD)
            s = l * 128
            wor[s : s + 64, :] = wo_r[:, Chh].T
            wor[s + 64 : s + 128, :] = -wo_i[:, Chh].T
            woi[s : s + 64, :] = wo_i[:, Chh].T
            woi[s + 64 : s + 128, :] = wo_r[:, Chh].T
        m["wor"] = wor.astype(BF16NP)
        m["woi"] = woi.astype(BF16NP)
        in_maps.append(m)

    # exact host-side bias: V-bias folds through softmax (rows sum to 1)
    cvr = (w["bv_r"] - w["bv_i"]).astype(np.float64)
    cvi = (w["bv_r"] + w["bv_i"]).astype(np.float64)
    wo_r = w["wo_r"].astype(np.float64)
    wo_i = w["wo_i"].astype(np.float64)
    br_total = wo_r @ cvr - wo_i @ cvi + w["bo_r"] - w["bo_i"]
    bi_total = wo_r @ cvi + wo_i @ cvr + w["bo_r"] + w["bo_i"]
    return in_maps, br_total.astype(np.float32), bi_total.astype(np.float32)


LAST_RESULTS = None


def kernel(**inputs):
    global LAST_RESULTS
    nc = _get_nc()
    in_maps, br_total, bi_total = _prep(inputs)
    res = run_bass_kernel_spmd(nc, in_maps, core_ids=list(range(N_CORES)))
    LAST_RESULTS = res
    out_r = np.zeros((B, L, D), np.float32)
    out_i = np.zeros((B, L, D), np.float32)
    for c in range(N_CORES):
        out_r[c // 4] += np.asarray(res.results[c]["pr"], np.float32)
        out_i[c // 4] += np.asarray(res.results[c]["pi"], np.float32)
    out_r += br_total[None, None, :]
    out_i += bi_total[None, None, :]
    return out_r, out_i


# revision 13
# speedup vs baseline: 1.7845x; 1.6427x over previous
"""ComplexMultiheadAttention on 8 Trainium2 NeuronCores.

Sharding: core c handles batch b = c//4 and the 4 heads [4*(c%4), 4*(c%4)+4).

v2 design:
- All tensor-engine operands in bf16 (same PE rate as f32r, half the bytes).
- Q/K projections and the O projection use Karatsuba complex multiplication:
  3 real matmuls (t1=zr@wr, t2=zi@wi, t3=(zr+zi)@(wr+wi)) instead of 4;
  real = t1-t2, imag = t3-t1-t2, combined on DVE/GpSimd (one PSUM operand is
  staged through a Scalar-engine copy because DVE ops may read only one PSUM
  input). The host ships zs = zr+zi and ws = wr+wi, so no extra on-chip adds.
- V projection stays as one fused K=2048 matmul over [zr|zi] (its seq-major
  output layout does not admit the 512-wide Karatsuba strips).
- Softmax rowsums ride DVE + GpSimd strip adds (f32 accumulation) with a
  single ones-matmul fold per (head, q-block); reciprocal uses the fast
  ~18-bit DVE approximation; exp is done once per kt-PAIR on [128,1024] PSUM.
- Phase 3 (O projection) is interleaved per q-block with phase 2 so the PE
  never drains; O-proj inputs are normalized AV tiles regrouped r/i via
  SBUF->SBUF DMA (DVE cannot shift partitions).
- Q-bias, K-bias applied in the combine; V-bias folds into the host-side
  output bias (softmax rows sum to 1); output partials are summed on host.
"""

import os
import sys

import numpy as np
import ml_dtypes

sys.path.insert(0, "/opt/trn_rl_repo")

import concourse.bass as bass
import concourse.bacc as bacc_mod
import concourse.mybir as mybir
from concourse.bass_utils import run_bass_kernel_spmd
from concourse.tile import TileContext

try:  # tracing needs antenv.axon_hooks (test harness injects it)
    import antenv.axon_hooks  # noqa: F401
except ImportError:
    os.environ.setdefault("BASS_NEVER_TRACE", "1")

B, L, D, NH = 2, 2048, 1024, 16
HD = D // NH  # 64
N_CORES = 8
NHL = 4  # heads per core
CH2 = 2 * NHL * HD  # 512 packed channels for fused V
F2 = 2 * D
SCALE = 1.0 / 8.0  # 1/sqrt(HD)
BF16NP = ml_dtypes.bfloat16

F32 = mybir.dt.float32
F32R = mybir.dt.float32r
BF16 = mybir.dt.bfloat16
AF = mybir.ActivationFunctionType
ALU = mybir.AluOpType

NSB = 4
SBW = L // NSB  # 512 seq cols per block


def _build_nc():
    nc = bacc_mod.Bacc(None, target_bir_lowering=False, debug=False)
    z3t = nc.declare_dram_parameter("z3t", [3 * D, L], BF16, isOutput=False)
    wdecl = {}
    for p in ("q", "k"):
        for t in (1, 2, 3):
            wdecl[f"w{p}{t}"] = nc.declare_dram_parameter(
                f"w{p}{t}", [D, 256], BF16, isOutput=False
            )
        for s in ("r", "i"):
            wdecl[f"c{p}{s}"] = nc.declare_dram_parameter(
                f"c{p}{s}", [256], F32, isOutput=False
            )
    wv = nc.declare_dram_parameter("wv", [F2, CH2], BF16, isOutput=False)
    wdecl["wor"] = nc.declare_dram_parameter("wor", [CH2, D], BF16, isOutput=False)
    wdecl["woi"] = nc.declare_dram_parameter("woi", [CH2, D], BF16, isOutput=False)
    pr = nc.declare_dram_parameter("pr", [L, D], BF16, isOutput=True)
    pi = nc.declare_dram_parameter("pi", [L, D], BF16, isOutput=True)

    with TileContext(nc) as tc:
        with tc.tile_pool(name="pers", bufs=1) as pers:
            # ---- persistent tiles ----
            qt = [
                pers.tile([128, L], BF16, tag=f"qt{h}", name=f"qt{h}")
                for h in range(NHL)
            ]
            kt = [
                pers.tile([128, L], BF16, tag=f"kt{h}", name=f"kt{h}")
                for h in range(NHL)
            ]
            v4 = pers.tile([128, 16, CH2], BF16, tag="v4")
            ones_f = pers.tile([128, 1], F32, tag="ones_f")
            nc.vector.memset(ones_f[:], 1.0)
            ones = pers.tile([128, 1], F32R, tag="ones")
            nc.scalar.activation(ones[:], ones_f[:], AF.Copy)
            onesr_f = pers.tile([1, 128], F32, tag="onesr_f")
            nc.vector.memset(onesr_f[:], 1.0)
            warm = pers.tile([128, 8], F32, tag="warm")

            # ---------- Phase 1: QKV projections ----------
            with (
                tc.tile_pool(name="w1p", bufs=1) as wpool,
                tc.tile_pool(name="z3p", bufs=2) as zpool,
                tc.tile_pool(name="st1", bufs=3) as st1,
                tc.tile_pool(name="ps_kar", bufs=2, space="PSUM") as ps_kar,
                tc.tile_pool(name="ps_v", bufs=2, space="PSUM") as ps_v,
            ):
                z3_first = zpool.tile([128, 24, SBW], BF16, tag="z3")

                def z3_dma(tile, sb):
                    for ftg in range(6):  # 6 DMAs of 4 chunks each
                        nc.sync.dma_start(
                            tile[:, ftg * 4 : (ftg + 1) * 4, :],
                            z3t[
                                ftg * 512 : (ftg + 1) * 512,
                                sb * SBW : (sb + 1) * SBW,
                            ].rearrange("(t p) s -> p t s", p=128),
                        )

                z3_dma(z3_first, 0)
                wsb = {}
                for p in ("q", "k"):
                    for t in (1, 2, 3):
                        wsb[f"{p}{t}"] = wpool.tile(
                            [128, 8, 256], BF16, tag=f"w{p}{t}", name=f"w{p}{t}"
                        )
                        nc.sync.dma_start(
                            wsb[f"{p}{t}"][:],
                            wdecl[f"w{p}{t}"][:].rearrange("(t p) s -> p t s", p=128),
                        )
                    for s in ("r", "i"):
                        wsb[f"c{p}{s}"] = wpool.tile(
                            [128, 2], F32, tag=f"c{p}{s}", name=f"c{p}{s}"
                        )
                        nc.sync.dma_start(
                            wsb[f"c{p}{s}"][:],
                            wdecl[f"c{p}{s}"][:].rearrange("(t p) -> p t", p=128),
                        )
                wv_sb = wpool.tile([128, 16, CH2], BF16, tag="wv")
                nc.sync.dma_start(
                    wv_sb[:], wv[:].rearrange("(t p) s -> p t s", p=128)
                )

                for sb in range(NSB):
                    if sb == 0:
                        z3_sb = z3_first
                    else:
                        z3_sb = zpool.tile([128, 24, SBW], BF16, tag="z3")
                        z3_dma(z3_sb, sb)
                    for p in ("q", "k"):
                        dst = qt if p == "q" else kt
                        for ct in range(2):
                            t1 = ps_kar.tile([128, SBW], F32, tag="t1")
                            t2 = ps_kar.tile([128, SBW], F32, tag="t2")
                            t3 = ps_kar.tile([128, SBW], F32, tag="t3")
                            for term, ps in ((1, t1), (2, t2), (3, t3)):
                                wt = wsb[f"{p}{term}"]
                                zoff = (term - 1) * 8
                                for ft in range(8):
                                    nc.tensor.matmul(
                                        ps[:],
                                        lhsT=wt[:, ft, ct * 128 : (ct + 1) * 128],
                                        rhs=z3_sb[:, zoff + ft, :],
                                        start=(ft == 0),
                                        stop=(ft == 7),
                                    )
                            t2_sb = st1.tile([128, SBW], F32R, tag="t2sb")
                            nc.scalar.activation(t2_sb[:], t2[:], AF.Copy)
                            cbr = wsb[f"c{p}r"][:, ct : ct + 1]
                            cbi = wsb[f"c{p}i"][:, ct : ct + 1]
                            reX = st1.tile([128, SBW], BF16, tag="reX")
                            u = st1.tile([128, SBW], F32R, tag="u")
                            imX = st1.tile([128, SBW], BF16, tag="imX")
                            with nc.allow_low_precision(reason="bf16 qkv"):
                                # real = (t1 + cbr) - t2
                                nc.vector.scalar_tensor_tensor(
                                    reX[:], t1[:], cbr, t2_sb[:],
                                    op0=ALU.add, op1=ALU.subtract,
                                )
                                # u = (t1 - cbi) + t2 ; imag = t3 - u
                                # (GpSimd cannot read PSUM, so all on DVE)
                                nc.vector.scalar_tensor_tensor(
                                    u[:], t1[:], cbi, t2_sb[:],
                                    op0=ALU.subtract, op1=ALU.add,
                                )
                                nc.vector.tensor_sub(imX[:], t3[:], u[:])
                            ha, hb = 2 * ct, 2 * ct + 1
                            cols = slice(sb * SBW, (sb + 1) * SBW)
                            nc.sync.dma_start(dst[ha][0:64, cols], reX[0:64, :])
                            nc.sync.dma_start(dst[hb][0:64, cols], reX[64:128, :])
                            nc.sync.dma_start(dst[ha][64:128, cols], imX[0:64, :])
                            nc.sync.dma_start(dst[hb][64:128, cols], imX[64:128, :])
                    for st in range(SBW // 128):
                        psv = ps_v.tile([128, CH2], F32, tag="psv")
                        for ft in range(16):
                            nc.tensor.matmul(
                                psv[:],
                                lhsT=z3_sb[:, ft, st * 128 : (st + 1) * 128],
                                rhs=wv_sb[:, ft, :],
                                start=(ft == 0),
                                stop=(ft == 15),
                            )
                        nc.scalar.activation(
                            v4[:, sb * 4 + st, :], psv[:], AF.Copy
                        )

            # warm up GpSimd custom-instruction library before phase 2
            nc.gpsimd.partition_broadcast(warm[:], onesr_f[0:1, 0:8])

            # ---------- Phase 2+3: attention with interleaved O-projection ----
            # Fused O-projection: OT tiles are the normalized AV tiles directly
            # (per-head [64r|64i] packing), no elementwise combine needed.
            with (
                tc.tile_pool(name="wo_p", bufs=1) as wopool,
                tc.tile_pool(name="otp", bufs=1) as otpool,
                tc.tile_pool(name="pp", bufs=12) as ppool,
                tc.tile_pool(name="tl1", bufs=2) as l1pool,
                tc.tile_pool(name="tl2", bufs=2) as l2pool,
                tc.tile_pool(name="tl3", bufs=2) as l3pool,
                tc.tile_pool(name="accs", bufs=2) as apool,
                tc.tile_pool(name="sm2", bufs=4) as spool,
                tc.tile_pool(name="st3", bufs=4) as st3,
                tc.tile_pool(name="ps_big", bufs=2, space="PSUM") as ps_big,
                tc.tile_pool(name="ps_av", bufs=2, space="PSUM") as ps_av,
                tc.tile_pool(name="ps_sum", bufs=2, space="PSUM") as ps_sum,
            ):
                wor_sb = wopool.tile([128, NHL, D], BF16, tag="wor")
                nc.sync.dma_start(
                    wor_sb[:], wdecl["wor"][:].rearrange("(t p) s -> p t s", p=128)
                )
                woi_sb = wopool.tile([128, NHL, D], BF16, tag="woi")
                nc.sync.dma_start(
                    woi_sb[:], wdecl["woi"][:].rearrange("(t p) s -> p t s", p=128)
                )
                # OT tiles: normalized AV, per (h, qb), [128 ch = 64r|64i, 512 q]
                ot = {}
                for qb in range(4):
                    for h in range(NHL):
                        ot[(h, qb)] = otpool.tile(
                            [128, SBW], BF16, tag=f"ot{h}_{qb}", name=f"ot{h}_{qb}"
                        )

                NAHEAD = 2  # pair lookahead
                p_tiles = {}

                def emit_pair(h, qb, kp):
                    sp = ps_big.tile([128, 1024], F32, tag="pair")
                    for j in range(2):
                        nc.tensor.matmul(
                            sp[:, j * 512 : (j + 1) * 512],
                            lhsT=kt[h][:, (2 * kp + j) * 128 : (2 * kp + j + 1) * 128],
                            rhs=qt[h][:, qb * 512 : (qb + 1) * 512],
                            start=True,
                            stop=True,
                        )
                    p_sb = ppool.tile([128, 1024], BF16, tag="p")
                    nc.scalar.activation(p_sb[:], sp[:], AF.Exp, scale=SCALE)
                    p_tiles[kp] = p_sb
                    return p_sb

                pending = []

                def flush_tail(limit):
                    while len(pending) > limit:
                        av, acc, h, qb = pending.pop(0)
                        ssum = ps_sum.tile([1, 512], F32, tag="ssum")
                        nc.tensor.matmul(
                            ssum[:], lhsT=ones[:, 0:1], rhs=acc[:],
                            start=True, stop=True,
                        )
                        recip = spool.tile([1, 512], F32, tag="recip")
                        nc.vector.reciprocal_approx_fast(recip[:], ssum[:])
                        rb = spool.tile([128, 512], F32, tag="rb")
                        nc.gpsimd.partition_broadcast(rb[:], recip[:])
                        with nc.allow_low_precision(reason="bf16 attn out"):
                            nc.vector.tensor_mul(ot[(h, qb)][:], av[:], rb[:])

                def attention_block(h, qb):
                    av = ps_av.tile([128, 512], F32, tag="av")
                    strips = []
                    # rowsum tree: all on DVE (GpSimd adds are 2.4x slower and
                    # would serialize the tail); issued incrementally so the
                    # final levels land right after the last exp.
                    l1 = []
                    l2 = []

                    def tree_l1(i):
                        t = l1pool.tile(
                            [128, 1024], BF16, tag=f"l1_{i}", name=f"l1_{i}"
                        )
                        with nc.allow_low_precision(reason="bf16 rowsum"):
                            nc.vector.tensor_add(
                                t[:], strips[2 * i][:], strips[2 * i + 1][:]
                            )
                        l1.append(t)

                    def tree_l2(i):
                        t = l2pool.tile(
                            [128, 1024], BF16, tag=f"l2_{i}", name=f"l2_{i}"
                        )
                        with nc.allow_low_precision(reason="bf16 rowsum"):
                            nc.vector.tensor_add(t[:], l1[2 * i][:], l1[2 * i + 1][:])
                        l2.append(t)

                    for kp in range(8):
                        if kp == 0:
                            for k2 in range(NAHEAD):
                                strips.append(emit_pair(h, qb, k2))
                        if kp + NAHEAD < 8:
                            strips.append(emit_pair(h, qb, kp + NAHEAD))
                        p_sb = p_tiles.pop(kp)
                        for j in range(2):
                            kt_i = 2 * kp + j
                            nc.tensor.matmul(
                                av[:],
                                lhsT=v4[:, kt_i, 128 * h : 128 * (h + 1)],
                                rhs=p_sb[:, j * 512 : (j + 1) * 512],
                                start=(kt_i == 0),
                                stop=(kt_i == 15),
                            )
                        if kp == 2:
                            tree_l1(0)
                        elif kp == 3:
                            flush_tail(0)
                        elif kp == 4:
                            tree_l1(1)
                        elif kp == 5:
                            tree_l2(0)
                        elif kp == 6:
                            tree_l1(2)
                    tree_l1(3)
                    tree_l2(1)
                    l3 = l3pool.tile([128, 1024], BF16, tag="l3")
                    acc = apool.tile([128, 512], F32R, tag="acc")
                    with nc.allow_low_precision(reason="bf16 rowsum"):
                        nc.vector.tensor_add(l3[:], l2[0][:], l2[1][:])
                        nc.vector.tensor_add(acc[:], l3[:, 0:512], l3[:, 512:1024])
                    pending.append((av, acc, h, qb))

                def oproj_group(qb, qt_local, nb):
                    big = ps_big.tile([128, 1024], F32, tag="pair")
                    rp = big[:, 0:512]
                    ip = big[:, 512:1024]
                    qs = slice(qt_local * 128, (qt_local + 1) * 128)
                    ns = slice(nb * 512, (nb + 1) * 512)
                    for h in range(NHL):
                        nc.tensor.matmul(
                            rp, lhsT=ot[(h, qb)][:, qs], rhs=wor_sb[:, h, ns],
                            start=(h == 0), stop=(h == NHL - 1),
                        )
                    for h in range(NHL):
                        nc.tensor.matmul(
                            ip, lhsT=ot[(h, qb)][:, qs], rhs=woi_sb[:, h, ns],
                            start=(h == 0), stop=(h == NHL - 1),
                        )
                    outr = st3.tile([128, 512], BF16, tag="outr")
                    nc.scalar.activation(outr[:], rp, AF.Copy)
                    outi = st3.tile([128, 512], BF16, tag="outi")
                    nc.scalar.activation(outi[:], ip, AF.Copy)
                    qrow = qb * 512 + qt_local * 128
                    nc.sync.dma_start(pr[qrow : qrow + 128, ns], outr[:])
                    nc.sync.dma_start(pi[qrow : qrow + 128, ns], outi[:])

                # schedule: run the NEXT qb's first attention block before each
                # O-proj batch so the PE has filler while the last head's
                # normalization tail completes.
                blocks = [(h2, qb2) for qb2 in range(4) for h2 in range(NHL)]
                bi = 0
                for qb in range(4):
                    target = (qb + 1) * NHL + (1 if qb < 3 else 0)
                    while bi < min(target, len(blocks)):
                        attention_block(*blocks[bi])
                        bi += 1
                    flush_tail(1 if qb < 3 else 0)
                    for qt_local in range(4):
                        for nb in range(2):
                            oproj_group(qb, qt_local, nb)
    if not nc.is_finalized():
        nc.finalize()
    return nc


_NC = None


def _get_nc():
    global _NC
    if _NC is None:
        _NC = _build_nc()
    return _NC


def _prep(inputs):
    f = lambda k: np.asarray(inputs[k], np.float32)
    zr, zi = f("zr"), f("zi")
    w = {n: f(n) for n in inputs if n not in ("zr", "zi")}

    z3t = []
    for b in range(B):
        zs = zr[b] + zi[b]
        z3t.append(
            np.concatenate([zr[b].T, zi[b].T, zs.T], axis=0).astype(BF16NP)
        )

    in_maps = []
    for c in range(N_CORES):
        b, hg = c // 4, c % 4
        m = {"z3t": z3t[b]}
        Ch = np.arange(hg * 256, (hg + 1) * 256)
        for name in ("q", "k"):
            wr = w[f"w{name}_r"][Ch, :]
            wi = w[f"w{name}_i"][Ch, :]
            m[f"w{name}1"] = np.ascontiguousarray(wr.T).astype(BF16NP)
            m[f"w{name}2"] = np.ascontiguousarray(wi.T).astype(BF16NP)
            m[f"w{name}3"] = np.ascontiguousarray((wr + wi).T).astype(BF16NP)
            br, bi = w[f"b{name}_r"][Ch], w[f"b{name}_i"][Ch]
            m[f"c{name}r"] = (br - bi).astype(np.float32)
            m[f"c{name}i"] = (br + bi).astype(np.float32)
        wvr, wvi = w["wv_r"], w["wv_i"]
        wvcat = np.empty((F2, CH2), np.float32)
        for l in range(NHL):
            Chh = np.arange((hg * 4 + l) * HD, (hg * 4 + l + 1) * HD)
            s = l * 128
            wvcat[:D, s : s + 64] = wvr[Chh, :].T
            wvcat[D:, s : s + 64] = -wvi[Chh, :].T
            wvcat[:D, s + 64 : s + 128] = wvi[Chh, :].T
            wvcat[D:, s + 64 : s + 128] = wvr[Chh, :].T
        m["wv"] = wvcat.astype(BF16NP)
        wo_r, wo_i = w["wo_r"], w["wo_i"]
        wor = np.empty((CH2, D), np.float32)
        woi = np.empty((CH2, D), np.float32)
        for l in range(NHL):
            Chh = np.arange((hg * 4 + l) * HD, (hg * 4 + l + 1) * HD)
            s = l * 128
            wor[s : s + 64, :] = wo_r[:, Chh].T
            wor[s + 64 : s + 128, :] = -wo_i[:, Chh].T
            woi[s : s + 64, :] = wo_i[:, Chh].T
            woi[s + 64 : s + 128, :] = wo_r[:, Chh].T
        m["wor"] = wor.astype(BF16NP)
        m["woi"] = woi.astype(BF16NP)
        in_maps.append(m)

    # exact host-side bias: V-bias folds through softmax (rows sum to 1)
    cvr = (w["bv_r"] - w["bv_i"]).astype(np.float64)
    cvi = (w["bv_r"] + w["bv_i"]).astype(np.float64)
    wo_r = w["wo_r"].astype(np.float64)
    wo_i = w["wo_i"].astype(np.float64)
    br_total = wo_r @ cvr - wo_i @ cvi + w["bo_r"] - w["bo_i"]
    bi_total = wo_r @ cvi + wo_i @ cvr + w["bo_r"] + w["bo_i"]
    return in_maps, br_total.astype(np.float32), bi_total.astype(np.float32)


LAST_RESULTS = None


def kernel(**inputs):
    global LAST_RESULTS
    nc = _get_nc()
    in_maps, br_total, bi_total = _prep(inputs)
    res = run_bass_kernel_spmd(nc, in_maps, core_ids=list(range(N_CORES)))
    LAST_RESULTS = res
    out_r = np.zeros((B, L, D), np.float32)
    out_i = np.zeros((B, L, D), np.float32)
    for c in range(N_CORES):
        out_r[c // 4] += np.asarray(res.results[c]["pr"], np.float32)
        out_i[c // 4] += np.asarray(res.results[c]["pi"], np.float32)
    out_r += br_total[None, None, :]
    out_i += bi_total[None, None, :]
    return out_r, out_i
